# revision 1
# baseline (speedup 1.0000x reference)
"""DeepSeek sparse attention on 8 Trainium2 NeuronCores (Bass/Tile).

Strategy (3 SPMD launches, column/head-parallel, float32r matmuls):

  L1  (column-parallel): each core computes a 256-column slice of the three
      projections, emitted transposed: qT/kT/vT slices (256, S) from
      hidden^T (resident in SBUF) and the core's weight column slice.
  host: concat slices -> q_lin^T, k_lin^T, v_lin^T (H, S).
  L2  (indexer-head-parallel): core c owns indexer head c. Computes
      qp_c^T, kp_c^T (256, S) from full q_lin^T / k_lin^T, then
      rel_c[q] = sum_k relu(qp_c[q] . kp_c[k]) via PE + fused relu-accum.
  host: rel = sum_c w_c * rel_c * exp(-T); top-1024 keys -> selected mask;
      hi[k] = selected ? BIG : k + LOCAL_WINDOW (fp16 threshold vector).
  L3  (attention-head-parallel): core c owns attention heads 2c, 2c+1.
      scores^T per head via PE (f32r), exp via ACT (fp16), causal/local/
      selected masking via two fused iota-compare-multiply DVE ops,
      denominator via ones-matmul, normalize, out rows = ao @ Wo[head rows]
      -> per-core partial (S, H).
  host: out = sum_c partial_c.

Matmuls run as float32r (full PE rate at N>=512, ~1.5e-4 rel err).
"""

import math

import numpy as np

import concourse.bass as bass
import concourse.mybir as mybir
from concourse import bacc
from concourse.tile import TileContext
from concourse.masks import make_identity
from concourse.bass_utils import run_bass_kernel_spmd

# Problem constants (hardcoded per contract)
HIDDEN = 2048
NUM_HEADS = 16
HEAD_DIM = 128
NUM_IND_HEADS = 8
IND_DIM = HIDDEN // NUM_IND_HEADS  # 256
MAX_SELECTED = 1024
LOCAL_WINDOW = 512
N_CORES = 8

F32 = mybir.dt.float32
F32R = mybir.dt.float32r
F16 = mybir.dt.float16
BF16 = mybir.dt.bfloat16
FP32 = np.float32

_TRACE = {"on": False, "exec_ns": []}


def _bc(ap):
    return ap.bitcast(F32R)


def build_l1(S=2048, H=HIDDEN, CS=HIDDEN // N_CORES):
    """Per-core: qT/kT/vT (CS, S) = (W[:, cols].T @ hidden.T) for 3 weights."""
    nc = bacc.Bacc("TRN2", target_bir_lowering=False, debug=False)
    HT, MC, NQ = H // 128, CS // 128, S // 512
    hidT = nc.dram_tensor("hidT", [H, S], F32R, kind="ExternalInput")
    wq = nc.dram_tensor("wq", [H, CS], F32R, kind="ExternalInput")
    wk = nc.dram_tensor("wk", [H, CS], F32R, kind="ExternalInput")
    wv = nc.dram_tensor("wv", [H, CS], F32R, kind="ExternalInput")
    qT = nc.dram_tensor("qT", [CS, S], F32, kind="ExternalOutput")
    kT = nc.dram_tensor("kT", [CS, S], F32, kind="ExternalOutput")
    vT = nc.dram_tensor("vT", [CS, S], F32, kind="ExternalOutput")

    with TileContext(nc) as tc:
        with (
            tc.tile_pool(name="hid", bufs=1) as hpool,
            tc.tile_pool(name="wt", bufs=4) as wpool,
            tc.tile_pool(name="ev", bufs=4) as opool,
            tc.tile_pool(name="ps", bufs=2, space="PSUM") as pspool,
        ):
            # hidden^T resident, loaded as 8 chunks of 2 k-strips so the first
            # matmuls only wait on chunk 0 (~2 MB), not the full 16 MB.
            G = 8
            TG = HT // G

            def load_hidc(g):
                hc = hpool.tile([128, TG * S], F32R, name=f"hidc{g}")
                nc.sync.dma_start(
                    out=hc.rearrange("p (t s) -> p t s", t=TG),
                    in_=hidT[g * TG * 128:(g + 1) * TG * 128, :].rearrange(
                        "(t p) s -> p t s", p=128
                    ),
                )
                return hc

            def load_wres(wdram):
                # weight column-slice resident: one 2 MB DMA per projection.
                wr = wpool.tile([128, HT * CS], F32R, tag="wres", name="wres")
                nc.sync.dma_start(
                    out=wr.rearrange("p (t c) -> p t c", t=HT),
                    in_=wdram.rearrange("(t p) c -> p t c", p=128),
                )
                return wr

            hidc = [load_hidc(0)]
            wres = {wq.name: load_wres(wq)}
            hidc += [load_hidc(g) for g in range(1, G)]
            wres[wk.name] = load_wres(wk)
            wres[wv.name] = load_wres(wv)

            for wdram, odram in ((wq, qT), (wk, kT), (wv, vT)):
                wr = wres[wdram.name]
                for mc in range(MC):
                    psums = [
                        pspool.tile([128, 512], F32, tag=f"ps{qc}", name=f"ps{qc}")
                        for qc in range(NQ)
                    ]
                    for t in range(HT):
                        lhsT = wr[:, t * CS + mc * 128: t * CS + mc * 128 + 128]
                        rhs_tile = hidc[t // TG]
                        tl = t % TG
                        for qc in range(NQ):
                            nc.tensor.matmul(
                                psums[qc], lhsT,
                                rhs_tile[:, tl * S + qc * 512: tl * S + qc * 512 + 512],
                                start=(t == 0), stop=(t == HT - 1),
                            )
                    for qc in range(NQ):
                        ot = opool.tile([128, 512], F32, tag="ot", name="ot")
                        nc.scalar.copy(ot, psums[qc])
                        nc.sync.dma_start(
                            out=odram[mc * 128:(mc + 1) * 128, qc * 512:(qc + 1) * 512],
                            in_=ot,
                        )
    nc.compile()
    return nc


def build_l2(S=2048, H=HIDDEN, D=IND_DIM):
    """Per-core (indexer head c): rel_c[q] = sum_k relu(qp_c[q] . kp_c[k])."""
    nc = bacc.Bacc("TRN2", target_bir_lowering=False, debug=False)
    HT, DC, NQ, QT = H // 128, D // 128, S // 512, S // 128
    qTd = nc.dram_tensor("qT", [H, S], F32R, kind="ExternalInput")
    kTd = nc.dram_tensor("kT", [H, S], F32R, kind="ExternalInput")
    wqi = nc.dram_tensor("wqi", [H, D], F32R, kind="ExternalInput")
    wki = nc.dram_tensor("wki", [H, D], F32R, kind="ExternalInput")
    rel = nc.dram_tensor("rel", [S], F32, kind="ExternalOutput")

    with TileContext(nc) as tc:
        with (
            tc.tile_pool(name="strip", bufs=3) as spool,
            tc.tile_pool(name="wstrip", bufs=3) as wpool,
            tc.tile_pool(name="proj", bufs=1) as ppool,
            tc.tile_pool(name="scr", bufs=3) as scpool,
            tc.tile_pool(name="rc", bufs=2) as rcpool,
            tc.tile_pool(name="rm", bufs=1) as rmpool,
            tc.tile_pool(name="ps", bufs=1, space="PSUM") as pspool,
        ):
            qpt = [ppool.tile([128, S], F32R, name=f"qpt{mc}") for mc in range(DC)]
            kpt = [ppool.tile([128, S], F32R, name=f"kpt{mc}") for mc in range(DC)]
            wires = {}
            for wd in (wqi, wki):
                wr = wpool.tile([128, HT * D], F32R, tag="wires", name="wires")
                nc.sync.dma_start(
                    out=wr.rearrange("p (t c) -> p t c", t=HT),
                    in_=wd.rearrange("(t p) c -> p t c", p=128),
                )
                wires[wd.name] = wr
            for xTd, wd, dst in ((qTd, wqi, qpt), (kTd, wki, kpt)):
                wr = wires[wd.name]
                psq = [
                    pspool.tile([128, 512], F32, tag=f"m{i}", name=f"m{i}")
                    for i in range(DC * NQ)
                ]
                for t in range(HT):
                    xs = spool.tile([128, S], F32R, tag="xs", name="xs")
                    nc.sync.dma_start(out=xs, in_=xTd[t * 128:(t + 1) * 128, :])
                    for mc in range(DC):
                        for qc in range(NQ):
                            nc.tensor.matmul(
                                psq[mc * NQ + qc],
                                wr[:, t * D + mc * 128: t * D + mc * 128 + 128],
                                xs[:, qc * 512:(qc + 1) * 512],
                                start=(t == 0), stop=(t == HT - 1),
                            )
                for mc in range(DC):
                    for qc in range(NQ):
                        nc.scalar.copy(
                            dst[mc][:, qc * 512:(qc + 1) * 512], psq[mc * NQ + qc]
                        )
            relmat = rmpool.tile([128, QT], F32, name="relmat")
            for qt in range(QT):
                relcols = rcpool.tile([128, NQ], F32, tag="relcols", name="relcols")
                spss = [
                    pspool.tile([128, 512], F32, tag=f"m{kc}", name="sps")
                    for kc in range(NQ)
                ]
                for d in range(DC):
                    for kc in range(NQ):
                        nc.tensor.matmul(
                            spss[kc],
                            qpt[d][:, qt * 128:(qt + 1) * 128],
                            kpt[d][:, kc * 512:(kc + 1) * 512],
                            start=(d == 0), stop=(d == DC - 1),
                        )
                for kc in range(NQ):
                    scratch = scpool.tile([128, 512], F16, tag="scratch", name="scratch")
                    nc.scalar.activation(
                        scratch, spss[kc], mybir.ActivationFunctionType.Relu,
                        accum_out=relcols[:, kc:kc + 1],
                    )
                nc.vector.tensor_reduce(
                    relmat[:, qt:qt + 1], relcols, axis=mybir.AxisListType.X,
                    op=mybir.AluOpType.add,
                )
            nc.sync.dma_start(
                out=rel.rearrange("(t p) -> p t", p=128), in_=relmat
            )
    nc.compile()
    return nc


def build_l3(S=2048, H=HIDDEN, NHC=NUM_HEADS // N_CORES, HD=HEAD_DIM,
             window=LOCAL_WINDOW):
    """Per-core (attention heads): partial (S, H) = sum_h softmax-attn @ Wo rows."""
    nc = bacc.Bacc("TRN2", target_bir_lowering=False, debug=False)
    KC, NQ, QT, OCC = S // 128, S // 512, S // 128, H // 512
    WT = window // 128  # local window in k-tiles
    qTh = nc.dram_tensor("qTh", [NHC * HD, S], F32R, kind="ExternalInput")
    kTh = nc.dram_tensor("kTh", [NHC * HD, S], F32R, kind="ExternalInput")
    vTh = nc.dram_tensor("vTh", [NHC * HD, S], F32, kind="ExternalInput")
    woh = nc.dram_tensor("woh", [NHC * HD, H], F32R, kind="ExternalInput")
    kidx = nc.dram_tensor("kidx", [S], F16, kind="ExternalInput")
    hivec = nc.dram_tensor("hivec", [S], F16, kind="ExternalInput")
    selv = nc.dram_tensor("selv", [S], F16, kind="ExternalInput")
    onesrow = nc.dram_tensor("onesrow", [128], F32R, kind="ExternalInput")
    part = nc.dram_tensor("part", [S, H], F32, kind="ExternalOutput")

    scale = 1.0 / math.sqrt(HD)
    AF = mybir.ActivationFunctionType
    OP = mybir.AluOpType

    with TileContext(nc) as tc:
        with (
            tc.tile_pool(name="const", bufs=1) as cpool,
            tc.tile_pool(name="qk", bufs=1) as qkpool,
            tc.tile_pool(name="vt", bufs=2) as vtpool,
            tc.tile_pool(name="vh", bufs=1) as vhpool,
            tc.tile_pool(name="vsl", bufs=1) as vslpool,
            tc.tile_pool(name="et", bufs=2) as etpool,
            tc.tile_pool(name="aon", bufs=1) as aopool,
            tc.tile_pool(name="dr", bufs=2) as drpool,
            tc.tile_pool(name="ev", bufs=4) as evpool,
            tc.tile_pool(name="ps", bufs=1, space="PSUM") as pspool,
        ):
            iota = cpool.tile([128, S], F16, name="iota")
            nc.gpsimd.iota(
                iota, pattern=[[1, S]], base=0, channel_multiplier=0,
                allow_small_or_imprecise_dtypes=True,
            )
            ones = cpool.tile([128, 1], F16, name="ones")
            nc.vector.memset(ones, 1.0)
            ident = cpool.tile([128, 128], F32, name="ident")
            make_identity(nc, ident)
            kvec = cpool.tile([128, KC], F16, name="kvec")
            nc.sync.dma_start(out=kvec, in_=kidx.rearrange("(t p) -> p t", p=128))
            hvec = cpool.tile([128, KC], F16, name="hvec")
            nc.sync.dma_start(out=hvec, in_=hivec.rearrange("(t p) -> p t", p=128))
            svec = cpool.tile([128, KC], F16, name="svec")
            nc.sync.dma_start(out=svec, in_=selv.rearrange("(t p) -> p t", p=128))
            svec32 = cpool.tile([128, KC], F32, name="svec32")
            nc.vector.tensor_copy(svec32, svec)
            ones1 = cpool.tile([1, 128], F32R, name="ones1")
            nc.sync.dma_start(out=ones1, in_=onesrow[None, :])

            # head-0 working set first so PE can start early; wo weights last.
            vts0 = vtpool.tile([128, S], F32, tag="vts", name="vts")
            nc.sync.dma_start(out=vts0, in_=vTh[0:HD, :])
            qsb, ksb = [], []
            for h in range(NHC):
                q = qkpool.tile([128, S], F32R, name=f"qsb{h}")
                nc.sync.dma_start(out=q, in_=qTh[h * HD:(h + 1) * HD, :])
                qsb.append(q)
                k = qkpool.tile([128, S], F32R, name=f"ksb{h}")
                nc.sync.dma_start(out=k, in_=kTh[h * HD:(h + 1) * HD, :])
                ksb.append(k)

            aon = [aopool.tile([128, S], F32R, name=f"aon{h}") for h in range(NHC)]
            vhf = [vhpool.tile([128, S], F16, name=f"vhf{h}") for h in range(NHC)]

            for h in range(NHC):
                if h == 0:
                    vts = vts0
                else:
                    vts = vtpool.tile([128, S], F32, tag="vts", name="vts")
                    nc.sync.dma_start(out=vts, in_=vTh[h * HD:(h + 1) * HD, :])
                for kc in range(KC):
                    tp = pspool.tile([128, 128], F32, tag="sc", bufs=3, name="tp")
                    nc.tensor.transpose(tp, vts[:, kc * 128:(kc + 1) * 128], ident)
                    nc.scalar.copy(vhf[h][:, kc * 128:(kc + 1) * 128], tp)
                # v pre-multiplied by the selected mask: beyond-local tiles use
                # it as the stationary operand, making masking free there.
                vsl = vslpool.tile([128, S], F16, name=f"vsl{h}")
                for kc in range(KC):
                    nc.vector.tensor_scalar_mul(
                        vsl[:, kc * 128:(kc + 1) * 128],
                        vhf[h][:, kc * 128:(kc + 1) * 128],
                        svec32[:, kc:kc + 1],
                    )
                # kc-outer: stationary operands (k tile, v tile) reused across
                # the q chunks; av/den accumulate per q chunk across kc.
                avp = [
                    pspool.tile([128, 512], F32, tag=f"av{qc}", bufs=1,
                                name=f"av{qc}")
                    for qc in range(NQ)
                ]
                den128 = pspool.tile([128, 512], F32, tag="den", bufs=1,
                                     name="den128")
                ets = {}
                for kc in range(KC):
                    k0 = kc * 128
                    qcs = [qc for qc in range(NQ) if k0 <= qc * 512 + 511]
                    far = {qc: qc * 512 > k0 + 127 + window for qc in qcs}
                    for qc in qcs:
                        q0 = qc * 512
                        q1 = q0 + 511
                        sps = pspool.tile([128, 512], F32, tag="sc", bufs=3,
                                          name="sps")
                        nc.tensor.matmul(
                            sps, ksb[h][:, kc * 128:(kc + 1) * 128],
                            qsb[h][:, q0:q0 + 512], start=True, stop=True,
                        )
                        et = etpool.tile([128, 512], F16, tag=f"et{qc}",
                                         name=f"et{qc}")
                        ets[qc] = et
                        nc.scalar.activation(et, sps, AF.Exp, scale=scale)
                        if far[qc]:
                            continue  # sel-mask folded into vsl/svec operands
                        if q0 < k0 + 128:
                            # causal: zero where q < k (iota - k < 0)
                            nc.gpsimd.affine_select(
                                out=et, in_=et, compare_op=OP.is_ge, fill=0.0,
                                base=q0 - k0, channel_multiplier=-1,
                                pattern=[[1, 512]],
                            )
                        if q1 > k0 + window:
                            nc.vector.scalar_tensor_tensor(
                                et, iota[:, q0:q0 + 512], hvec[:, kc:kc + 1], et,
                                op0=OP.is_le, op1=OP.mult,
                            )
                    for qc in qcs:
                        lhs_av = vsl if far[qc] else vhf[h]
                        nc.tensor.matmul(
                            avp[qc], lhs_av[:, kc * 128:(kc + 1) * 128], ets[qc],
                            start=(kc == 0), stop=(kc == (qc * 512 + 511) // 128),
                        )
                    for qc in qcs:
                        lhs_den = svec[:, kc:kc + 1] if far[qc] else ones
                        nc.tensor.matmul(
                            den128[32 * qc:32 * qc + 1, :], lhs_den, ets[qc],
                            start=(kc == 0), stop=(kc == (qc * 512 + 511) // 128),
                            tile_position=(0, 32 * qc),
                        )
                # denominators -> reciprocals -> broadcast -> normalize
                for qc in range(NQ):
                    q0 = qc * 512
                    dq = drpool.tile([1, 512], F32, tag=f"dq{qc}", name=f"dq{qc}")
                    nc.scalar.copy(dq, den128[32 * qc:32 * qc + 1, :])
                    rq = drpool.tile([1, 512], F32, tag=f"rq{qc}", name=f"rq{qc}")
                    rs = drpool.tile([1, 512], F32, tag=f"rs{qc}", name=f"rs{qc}")
                    nc.vector.reciprocal_approx_accurate(rq, dq, rs)
                    rcq = drpool.tile([1, 512], F32R, tag=f"rcq{qc}",
                                      name=f"rcq{qc}")
                    nc.vector.tensor_copy(rcq, rq)
                    rb = pspool.tile([128, 512], F32, tag="sc", bufs=3, name="rb")
                    nc.tensor.matmul(rb, ones1, rcq, start=True, stop=True)
                    rbs = drpool.tile([128, 512], F32, tag="rbs", name="rbs")
                    nc.scalar.copy(rbs, rb)
                    nc.vector.scalar_tensor_tensor(
                        aon[h][:, q0:q0 + 512], rbs, 1.0, avp[qc],
                        op0=OP.mult, op1=OP.mult,
                    )
            wsb = []
            for h in range(NHC):
                w = qkpool.tile([128, H], F32R, name=f"wsb{h}")
                nc.sync.dma_start(out=w, in_=woh[h * HD:(h + 1) * HD, :])
                wsb.append(w)
            for qt in range(QT):
                wops = [
                    pspool.tile([128, 512], F32, tag=f"av{oc}", bufs=1,
                                name=f"wops{oc}")
                    for oc in range(OCC)
                ]
                for h in range(NHC):
                    for oc in range(OCC):
                        nc.tensor.matmul(
                            wops[oc], aon[h][:, qt * 128:(qt + 1) * 128],
                            wsb[h][:, oc * 512:(oc + 1) * 512],
                            start=(h == 0), stop=(h == NHC - 1),
                        )
                for oc in range(OCC):
                    ot = evpool.tile([128, 512], F32, tag="ot", name="ot")
                    nc.vector.tensor_copy(ot, wops[oc])
                    nc.sync.dma_start(
                        out=part[qt * 128:(qt + 1) * 128, oc * 512:(oc + 1) * 512],
                        in_=ot,
                    )
    nc.compile()
    return nc


_CACHE = {}


def _get(name, builder, *args):
    key = (name,) + args
    if key not in _CACHE:
        _CACHE[key] = builder(*args)
    return _CACHE[key]


def _run(nc, in_maps):
    res = run_bass_kernel_spmd(
        nc, in_maps, core_ids=list(range(N_CORES)), trace=_TRACE["on"]
    )
    if _TRACE["on"] and res.exec_time_ns is not None:
        _TRACE["exec_ns"].append(res.exec_time_ns)
    return res.results


def kernel(hidden_states, Wq, Wk, Wv, Wo, Wq_ind, Wk_ind, head_weights,
           temperature_param):
    hidden_states = np.asarray(hidden_states, dtype=FP32)
    Wq, Wk, Wv, Wo = (np.asarray(a, dtype=FP32) for a in (Wq, Wk, Wv, Wo))
    Wq_ind = np.asarray(Wq_ind, dtype=FP32)
    Wk_ind = np.asarray(Wk_ind, dtype=FP32)
    head_weights = np.asarray(head_weights, dtype=FP32)
    temp = float(np.asarray(temperature_param))

    B, S, H = hidden_states.shape
    assert B == 1 and H == HIDDEN
    CS = H // N_CORES
    hidT = np.ascontiguousarray(hidden_states[0].T)

    # ---- L1: projections, column-parallel ----
    nc1 = _get("l1", build_l1, S, H, CS)
    in1 = [
        {
            "hidT": hidT,
            "wq": np.ascontiguousarray(Wq[:, c * CS:(c + 1) * CS]),
            "wk": np.ascontiguousarray(Wk[:, c * CS:(c + 1) * CS]),
            "wv": np.ascontiguousarray(Wv[:, c * CS:(c + 1) * CS]),
        }
        for c in range(N_CORES)
    ]
    r1 = _run(nc1, in1)
    qTf = np.concatenate([r["qT"] for r in r1], axis=0)
    kTf = np.concatenate([r["kT"] for r in r1], axis=0)
    vTf = np.concatenate([r["vT"] for r in r1], axis=0)

    # ---- L2: lightning indexer, head-parallel ----
    D = IND_DIM
    nc2 = _get("l2", build_l2, S, H, D)
    in2 = [
        {
            "qT": qTf,
            "kT": kTf,
            "wqi": np.ascontiguousarray(Wq_ind[:, c * D:(c + 1) * D]),
            "wki": np.ascontiguousarray(Wk_ind[:, c * D:(c + 1) * D]),
        }
        for c in range(N_CORES)
    ]
    r2 = _run(nc2, in2)
    rel = np.zeros(S, dtype=np.float64)
    for c in range(N_CORES):
        rel += float(head_weights[c]) * r2[c]["rel"].astype(np.float64)
    # exp(-temp) scaling is monotone; irrelevant for top-k selection.

    k_sel = min(MAX_SELECTED, S)
    top_idx = np.argpartition(-rel, k_sel - 1)[:k_sel]
    selected = np.zeros(S, dtype=bool)
    selected[top_idx] = True

    # ---- L3: masked attention + output projection, head-parallel ----
    BIG = float(2 * S + 1024)
    hi = np.where(selected, BIG, np.arange(S, dtype=np.float64) + LOCAL_WINDOW)
    hi = hi.astype(np.float16)
    kidx = np.arange(S, dtype=np.float16)
    selv = selected.astype(np.float16)
    NHC = NUM_HEADS // N_CORES
    RW = NHC * HEAD_DIM
    nc3 = _get("l3", build_l3, S, H, NHC, HEAD_DIM, LOCAL_WINDOW)
    in3 = [
        {
            "qTh": np.ascontiguousarray(qTf[c * RW:(c + 1) * RW]),
            "kTh": np.ascontiguousarray(kTf[c * RW:(c + 1) * RW]),
            "vTh": np.ascontiguousarray(vTf[c * RW:(c + 1) * RW]),
            "woh": np.ascontiguousarray(Wo[c * RW:(c + 1) * RW]),
            "kidx": kidx,
            "hivec": hi,
            "selv": selv,
            "onesrow": np.ones(128, dtype=np.float32),
        }
        for c in range(N_CORES)
    ]
    r3 = _run(nc3, in3)
    out = r3[0]["part"]
    for c in range(1, N_CORES):
        out = out + r3[c]["part"]
    return out.reshape(B, S, H).astype(np.float32)



# revision 3
# speedup vs baseline: 1.4837x; 1.4837x over previous
"""DeepSeek sparse attention on 8 Trainium2 NeuronCores (Bass/Tile), v2.

Two SPMD launches (down from three):

  A   (column/indexer-head-parallel): core c computes the 256-col slices
      of the q/k/v projections (emitted transposed, bf16/f16) AND its
      indexer head's relevance scores rel_c using HOST-FUSED indexer
      weights (Wq@Wq_ind, Wk@Wk_ind).  The fusion decouples the indexer
      from q_lin/k_lin, killing the baseline's launch 2 (which reloaded
      32MB/core of qT/kT).  All matmul inputs bf16 (same PE rate as
      f32r, half the DMA).  PE order: indexer projections first, then
      indexer score groups interleaved into the q/k/v matmul stream so
      ACT relu latency never stalls PE.
  host: rel = sum_c w_c*rel_c; top-1024 -> selected mask; v transposed
      to key-major f16 and premultiplied by the mask; hi threshold vec.
  B   (attention-head-parallel): core c owns heads 2c, 2c+1: softmax
      attention with causal/local/selected masking + output-projection
      partial (f16).  PE issue order software-pipelined: scores of key
      tile kc+1 are issued before AV of kc, hiding exp/mask latency.
  host: out = sum_c partial_c.
"""

import math

import numpy as np
import ml_dtypes

import concourse.bass as bass
import concourse.mybir as mybir
from concourse import bacc
from concourse.tile import TileContext
from concourse.bass_utils import run_bass_kernel_spmd

# Problem constants (hardcoded per contract)
HIDDEN = 2048
NUM_HEADS = 16
HEAD_DIM = 128
NUM_IND_HEADS = 8
IND_DIM = HIDDEN // NUM_IND_HEADS  # 256
MAX_SELECTED = 1024
LOCAL_WINDOW = 512
N_CORES = 8
SEQ = 2048

F32 = mybir.dt.float32
F32R = mybir.dt.float32r
F16 = mybir.dt.float16
BF16 = mybir.dt.bfloat16
NP_BF16 = ml_dtypes.bfloat16
FP32 = np.float32

_TRACE = {"on": False, "exec_ns": []}


def build_la(S=SEQ, H=HIDDEN, CS=HIDDEN // N_CORES):
    """Per-core: qT/kT/vT (CS, S) slices + indexer-head rel (S)."""
    nc = bacc.Bacc("TRN2", target_bir_lowering=False, debug=False)
    HT, MC, NQ, QT, DC = H // 128, CS // 128, S // 512, S // 128, IND_DIM // 128
    hidT = nc.dram_tensor("hidT", [H, S], BF16, kind="ExternalInput")
    wq = nc.dram_tensor("wq", [H, CS], BF16, kind="ExternalInput")
    wk = nc.dram_tensor("wk", [H, CS], BF16, kind="ExternalInput")
    wv = nc.dram_tensor("wv", [H, CS], BF16, kind="ExternalInput")
    wqi = nc.dram_tensor("wqi", [H, CS], BF16, kind="ExternalInput")
    wki = nc.dram_tensor("wki", [H, CS], BF16, kind="ExternalInput")
    qT = nc.dram_tensor("qT", [CS, S], BF16, kind="ExternalOutput")
    kT = nc.dram_tensor("kT", [CS, S], BF16, kind="ExternalOutput")
    vT = nc.dram_tensor("vT", [CS, S], F16, kind="ExternalOutput")
    rel = nc.dram_tensor("rel", [S], F32, kind="ExternalOutput")

    AF = mybir.ActivationFunctionType
    OP = mybir.AluOpType

    with TileContext(nc) as tc:
        with (
            tc.tile_pool(name="hid", bufs=1) as hpool,
            tc.tile_pool(name="wt", bufs=1) as wpool,
            tc.tile_pool(name="proj", bufs=1) as ppool,
            tc.tile_pool(name="ev", bufs=2) as opool,
            tc.tile_pool(name="scr", bufs=2) as scpool,
            tc.tile_pool(name="rc", bufs=2) as rcpool,
            tc.tile_pool(name="rm", bufs=1) as rmpool,
            tc.tile_pool(name="ps", bufs=1, space="PSUM") as pspool,
        ):
            def load_w(wdram):
                wr = wpool.tile([128, HT * CS], BF16, name=f"w_{wdram.name}")
                nc.sync.dma_start(
                    out=wr.rearrange("p (t c) -> p t c", t=HT),
                    in_=wdram.rearrange("(t p) c -> p t c", p=128),
                )
                return wr

            def load_strip(t):
                hs = hpool.tile([128, S], BF16, name=f"hid{t}")
                nc.sync.dma_start(out=hs, in_=hidT[t * 128:(t + 1) * 128, :])
                return hs

            # DMA order: first operands for the indexer projections, then the
            # rest of hidden, then q/k/v weights (needed ~55us in).
            wqi_t = load_w(wqi)
            hids = [load_strip(0)]
            wki_t = load_w(wki)
            hids += [load_strip(t) for t in range(1, HT)]
            wq_t, wk_t, wv_t = load_w(wq), load_w(wk), load_w(wv)

            qpi = [ppool.tile([128, S], BF16, name=f"qpi{d}") for d in range(DC)]
            kpi = [ppool.tile([128, S], BF16, name=f"kpi{d}") for d in range(DC)]

            # ---- phase 1: indexer projections (hidden @ fused weights) ----
            for wt, dst in ((wqi_t, qpi), (wki_t, kpi)):
                for mc in range(MC):
                    psums = [
                        pspool.tile([128, 512], F32, tag=f"p{qc}", name=f"p{qc}")
                        for qc in range(NQ)
                    ]
                    for t in range(HT):
                        lhsT = wt[:, t * CS + mc * 128: t * CS + mc * 128 + 128]
                        for qc in range(NQ):
                            nc.tensor.matmul(
                                psums[qc], lhsT,
                                hids[t][:, qc * 512:(qc + 1) * 512],
                                start=(t == 0), stop=(t == HT - 1),
                            )
                    for qc in range(NQ):
                        nc.vector.tensor_copy(
                            dst[mc][:, qc * 512:(qc + 1) * 512], psums[qc]
                        )

            # ---- phase 2: q/k/v projections interleaved with score groups ----
            relmat = rmpool.tile([128, QT], F32, name="relmat")

            def gen_qkv():
                for wt, odram, odt in (
                    (wq_t, qT, BF16), (wk_t, kT, BF16), (wv_t, vT, F16),
                ):
                    for mc in range(MC):
                        psums = [
                            pspool.tile([128, 512], F32, tag=f"p{qc}",
                                        name=f"pp{qc}")
                            for qc in range(NQ)
                        ]
                        for t in range(HT):
                            lhsT = wt[:, t * CS + mc * 128:
                                      t * CS + mc * 128 + 128]
                            for qc in range(NQ):
                                nc.tensor.matmul(
                                    psums[qc], lhsT,
                                    hids[t][:, qc * 512:(qc + 1) * 512],
                                    start=(t == 0), stop=(t == HT - 1),
                                )
                            if t == 7:
                                yield
                        for qc in range(NQ):
                            ot = opool.tile([128, 512], odt, tag=f"ot{qc}",
                                            name=f"ot{qc}")
                            nc.vector.tensor_copy(ot, psums[qc])
                            nc.sync.dma_start(
                                out=odram[mc * 128:(mc + 1) * 128,
                                          qc * 512:(qc + 1) * 512],
                                in_=ot,
                            )
                        yield

            qkv = gen_qkv()
            for qt in range(QT):
                # score group qt: rel rows for q-tile qt
                sps = [
                    pspool.tile([128, 512], F32, tag=f"s{kc}", name=f"s{kc}")
                    for kc in range(NQ)
                ]
                for kc in range(NQ):
                    for d in range(DC):
                        nc.tensor.matmul(
                            sps[kc],
                            qpi[d][:, qt * 128:(qt + 1) * 128],
                            kpi[d][:, kc * 512:(kc + 1) * 512],
                            start=(d == 0), stop=(d == DC - 1),
                        )
                relcols = rcpool.tile([128, NQ], F32, tag="relcols",
                                      name="relcols")
                for kc in range(NQ):
                    scratch = scpool.tile([128, 512], F16, tag="scratch",
                                          name="scratch")
                    nc.scalar.activation(
                        scratch, sps[kc], AF.Relu,
                        accum_out=relcols[:, kc:kc + 1],
                    )
                nc.vector.tensor_reduce(
                    relmat[:, qt:qt + 1], relcols, axis=mybir.AxisListType.X,
                    op=OP.add,
                )
                next(qkv, None)
            for _ in qkv:
                pass

            nc.sync.dma_start(
                out=rel.rearrange("(t p) -> p t", p=128), in_=relmat
            )
    nc.compile()
    return nc


def build_lb(S=SEQ, H=HIDDEN, NHC=NUM_HEADS // N_CORES, HD=HEAD_DIM,
             window=LOCAL_WINDOW):
    """Per-core (attention heads): partial (S, H) f16 = softmax-attn @ Wo rows."""
    nc = bacc.Bacc("TRN2", target_bir_lowering=False, debug=False)
    KC, NQ, QT, OCC = S // 128, S // 512, S // 128, H // 512
    qTh = nc.dram_tensor("qTh", [NHC * HD, S], BF16, kind="ExternalInput")
    kTh = nc.dram_tensor("kTh", [NHC * HD, S], BF16, kind="ExternalInput")
    vh = nc.dram_tensor("vh", [S, NHC * HD], F16, kind="ExternalInput")
    vslh = nc.dram_tensor("vslh", [S, NHC * HD], F16, kind="ExternalInput")
    woh = nc.dram_tensor("woh", [NHC * HD, H], BF16, kind="ExternalInput")
    hivec = nc.dram_tensor("hivec", [S], F16, kind="ExternalInput")
    selv = nc.dram_tensor("selv", [S], F16, kind="ExternalInput")
    part = nc.dram_tensor("part", [S, H], F16, kind="ExternalOutput")

    scale = 1.0 / math.sqrt(HD)
    AF = mybir.ActivationFunctionType
    OP = mybir.AluOpType

    with TileContext(nc) as tc:
        with (
            tc.tile_pool(name="const", bufs=1) as cpool,
            tc.tile_pool(name="qk", bufs=1) as qkpool,
            tc.tile_pool(name="vv", bufs=1) as vpool,
            tc.tile_pool(name="et", bufs=2) as etpool,
            tc.tile_pool(name="aon", bufs=1) as aopool,
            tc.tile_pool(name="dr", bufs=2) as drpool,
            tc.tile_pool(name="ev", bufs=2) as evpool,
            tc.tile_pool(name="ps", bufs=1, space="PSUM") as pspool,
        ):
            # head-0 q/k first so PE can start early
            qsb, ksb = [], []
            for h in range(NHC):
                q = qkpool.tile([128, S], BF16, name=f"qsb{h}")
                nc.sync.dma_start(out=q, in_=qTh[h * HD:(h + 1) * HD, :])
                qsb.append(q)
                k = qkpool.tile([128, S], BF16, name=f"ksb{h}")
                nc.sync.dma_start(out=k, in_=kTh[h * HD:(h + 1) * HD, :])
                ksb.append(k)
            # v / v*sel in key-major layout direct from HBM (no transposes)
            vhf, vsl = [], []
            for h in range(NHC):
                vt = vpool.tile([128, KC * HD], F16, name=f"vhf{h}")
                nc.sync.dma_start(
                    out=vt.rearrange("p (t d) -> p t d", t=KC),
                    in_=vh[:, h * HD:(h + 1) * HD].rearrange(
                        "(t p) d -> p t d", p=128),
                )
                vhf.append(vt)
                vs = vpool.tile([128, KC * HD], F16, name=f"vsl{h}")
                nc.sync.dma_start(
                    out=vs.rearrange("p (t d) -> p t d", t=KC),
                    in_=vslh[:, h * HD:(h + 1) * HD].rearrange(
                        "(t p) d -> p t d", p=128),
                )
                vsl.append(vs)

            iota = cpool.tile([128, S], F16, name="iota")
            nc.gpsimd.iota(
                iota, pattern=[[1, S]], base=0, channel_multiplier=0,
                allow_small_or_imprecise_dtypes=True,
            )
            ones = cpool.tile([128, 1], F16, name="ones")
            nc.vector.memset(ones, 1.0)
            ones1 = cpool.tile([1, 128], F16, name="ones1")
            nc.vector.memset(ones1, 1.0)
            hvec = cpool.tile([128, KC], F16, name="hvec")
            nc.sync.dma_start(out=hvec, in_=hivec.rearrange("(t p) -> p t", p=128))
            svec = cpool.tile([128, KC], F16, name="svec")
            nc.sync.dma_start(out=svec, in_=selv.rearrange("(t p) -> p t", p=128))

            wsb = []
            for h in range(NHC):
                w = qkpool.tile([128, H], BF16, name=f"wsb{h}")
                nc.sync.dma_start(out=w, in_=woh[h * HD:(h + 1) * HD, :])
                wsb.append(w)

            aon = [aopool.tile([128, S], BF16, name=f"aon{h}")
                   for h in range(NHC)]

            for h in range(NHC):
                avp = [
                    pspool.tile([128, 512], F32, tag=f"av{qc}", bufs=1,
                                name=f"av{qc}")
                    for qc in range(NQ)
                ]
                den128 = pspool.tile([128, 512], F32, tag="den", bufs=1,
                                     name="den128")

                def emit_av_den(kc, qcs, far, ets):
                    for qc in qcs:
                        lhs_av = vsl[h] if far[qc] else vhf[h]
                        nc.tensor.matmul(
                            avp[qc], lhs_av[:, kc * 128:(kc + 1) * 128],
                            ets[qc], start=(kc == 0),
                            stop=(kc == (qc * 512 + 511) // 128),
                        )
                    for qc in qcs:
                        lhs_den = svec[:, kc:kc + 1] if far[qc] else ones
                        nc.tensor.matmul(
                            den128[32 * qc:32 * qc + 1, :], lhs_den, ets[qc],
                            start=(kc == 0),
                            stop=(kc == (qc * 512 + 511) // 128),
                            tile_position=(0, 32 * qc),
                        )

                pend = None
                for kc in range(KC):
                    k0 = kc * 128
                    qcs = [qc for qc in range(NQ) if qc * 512 + 511 >= k0]
                    far = {qc: qc * 512 > k0 + 127 + window for qc in qcs}
                    ets = {}
                    for qc in qcs:
                        q0 = qc * 512
                        sps = pspool.tile([128, 512], F32, tag="sc", bufs=3,
                                          name="sps")
                        nc.tensor.matmul(
                            sps, ksb[h][:, k0:k0 + 128],
                            qsb[h][:, q0:q0 + 512], start=True, stop=True,
                        )
                        et = etpool.tile([128, 512], F16, tag=f"et{qc}",
                                         name=f"et{qc}")
                        ets[qc] = et
                        nc.scalar.activation(et, sps, AF.Exp, scale=scale)
                        if far[qc]:
                            continue  # sel-mask folded into vsl/svec operands
                        if q0 < k0 + 128:
                            # causal: zero where q < k
                            nc.gpsimd.affine_select(
                                out=et, in_=et, compare_op=OP.is_ge, fill=0.0,
                                base=q0 - k0, channel_multiplier=-1,
                                pattern=[[1, 512]],
                            )
                        if q0 + 511 > k0 + window:
                            nc.vector.scalar_tensor_tensor(
                                et, iota[:, q0:q0 + 512], hvec[:, kc:kc + 1],
                                et, op0=OP.is_le, op1=OP.mult,
                            )
                    if pend is not None:
                        emit_av_den(*pend)
                    pend = (kc, qcs, far, ets)
                emit_av_den(*pend)

                # denominators -> reciprocals -> broadcast -> normalize
                for qc in range(NQ):
                    q0 = qc * 512
                    dq = drpool.tile([1, 512], F32, tag=f"dq{qc}",
                                     name=f"dq{qc}")
                    nc.scalar.copy(dq, den128[32 * qc:32 * qc + 1, :])
                    rq = drpool.tile([1, 512], F32, tag=f"rq{qc}",
                                     name=f"rq{qc}")
                    rs = drpool.tile([1, 512], F32, tag=f"rs{qc}",
                                     name=f"rs{qc}")
                    nc.vector.reciprocal_approx_accurate(rq, dq, rs)
                    rcq = drpool.tile([1, 512], F16, tag=f"rcq{qc}",
                                      name=f"rcq{qc}")
                    nc.vector.tensor_copy(rcq, rq)
                    rb = pspool.tile([128, 512], F32, tag="sc", bufs=3,
                                     name="rb")
                    nc.tensor.matmul(rb, ones1, rcq, start=True, stop=True)
                    rbs = drpool.tile([128, 512], F32, tag="rbs", name="rbs")
                    nc.scalar.copy(rbs, rb)
                    nc.vector.scalar_tensor_tensor(
                        aon[h][:, q0:q0 + 512], rbs, 1.0, avp[qc],
                        op0=OP.mult, op1=OP.mult,
                    )

            # output projection: partial = sum_h aon_h @ Wo rows
            nev = 0
            for qt in range(QT):
                wops = [
                    pspool.tile([128, 512], F32, tag=f"av{oc}", bufs=1,
                                name=f"wops{oc}")
                    for oc in range(OCC)
                ]
                for h in range(NHC):
                    for oc in range(OCC):
                        nc.tensor.matmul(
                            wops[oc], aon[h][:, qt * 128:(qt + 1) * 128],
                            wsb[h][:, oc * 512:(oc + 1) * 512],
                            start=(h == 0), stop=(h == NHC - 1),
                        )
                for oc in range(OCC):
                    ot = evpool.tile([128, 512], F16, tag=f"ot{oc}",
                                     name=f"ot{oc}")
                    eng = nev % 2
                    nev += 1
                    if eng == 0:
                        nc.scalar.copy(ot, wops[oc])
                    else:
                        nc.vector.tensor_copy(ot, wops[oc])
                    nc.sync.dma_start(
                        out=part[qt * 128:(qt + 1) * 128,
                                 oc * 512:(oc + 1) * 512],
                        in_=ot,
                    )
    nc.compile()
    return nc


_CACHE = {}


def _get(name, builder, *args):
    key = (name,) + args
    if key not in _CACHE:
        _CACHE[key] = builder(*args)
    return _CACHE[key]


def _run(nc, in_maps):
    res = run_bass_kernel_spmd(
        nc, in_maps, core_ids=list(range(N_CORES)), trace=_TRACE["on"]
    )
    if _TRACE["on"] and res.exec_time_ns is not None:
        _TRACE["exec_ns"].append(res.exec_time_ns)
    return res.results


def kernel(hidden_states, Wq, Wk, Wv, Wo, Wq_ind, Wk_ind, head_weights,
           temperature_param):
    hidden_states = np.asarray(hidden_states, dtype=FP32)
    Wq, Wk, Wv, Wo = (np.asarray(a, dtype=FP32) for a in (Wq, Wk, Wv, Wo))
    Wq_ind = np.asarray(Wq_ind, dtype=FP32)
    Wk_ind = np.asarray(Wk_ind, dtype=FP32)
    head_weights = np.asarray(head_weights, dtype=FP32)

    B, S, H = hidden_states.shape
    assert B == 1 and H == HIDDEN and S == SEQ
    CS = H // N_CORES

    # fused indexer weights: qp = q_lin@Wq_ind = hidden@(Wq@Wq_ind)
    Wqi_f = Wq @ Wq_ind
    Wki_f = Wk @ Wk_ind

    hidT = np.ascontiguousarray(hidden_states[0].T).astype(NP_BF16)

    # ---- launch A: projections + indexer rel ----
    nca = _get("la", build_la, S, H, CS)
    ina = [
        {
            "hidT": hidT,
            "wq": np.ascontiguousarray(Wq[:, c * CS:(c + 1) * CS]).astype(NP_BF16),
            "wk": np.ascontiguousarray(Wk[:, c * CS:(c + 1) * CS]).astype(NP_BF16),
            "wv": np.ascontiguousarray(Wv[:, c * CS:(c + 1) * CS]).astype(NP_BF16),
            "wqi": np.ascontiguousarray(Wqi_f[:, c * CS:(c + 1) * CS]).astype(NP_BF16),
            "wki": np.ascontiguousarray(Wki_f[:, c * CS:(c + 1) * CS]).astype(NP_BF16),
        }
        for c in range(N_CORES)
    ]
    ra = _run(nca, ina)

    rel = np.zeros(S, dtype=np.float64)
    for c in range(N_CORES):
        rel += float(head_weights[c]) * np.asarray(ra[c]["rel"], dtype=np.float64)
    # exp(-temp) scaling is monotone; irrelevant for top-k selection.

    k_sel = min(MAX_SELECTED, S)
    top_idx = np.argpartition(-rel, k_sel - 1)[:k_sel]
    selected = np.zeros(S, dtype=bool)
    selected[top_idx] = True

    # ---- launch B: masked attention + output projection ----
    BIG = float(2 * S + 1024)
    hi = np.where(selected, BIG, np.arange(S, dtype=np.float64) + LOCAL_WINDOW)
    hi = hi.astype(np.float16)
    selv = selected.astype(np.float16)
    NHC = NUM_HEADS // N_CORES
    RW = NHC * HEAD_DIM

    ncb = _get("lb", build_lb, S, H, NHC, HEAD_DIM, LOCAL_WINDOW)
    inb = []
    for c in range(N_CORES):
        vhc = np.ascontiguousarray(
            np.asarray(ra[c]["vT"], dtype=np.float16).T)  # (S, 256) key-major
        inb.append({
            "qTh": np.asarray(ra[c]["qT"]),
            "kTh": np.asarray(ra[c]["kT"]),
            "vh": vhc,
            "vslh": np.ascontiguousarray(vhc * selv[:, None]),
            "woh": np.ascontiguousarray(Wo[c * RW:(c + 1) * RW]).astype(NP_BF16),
            "hivec": hi,
            "selv": selv,
        })
    rb = _run(ncb, inb)
    out = np.zeros((S, H), dtype=np.float32)
    for c in range(N_CORES):
        out += np.asarray(rb[c]["part"], dtype=np.float32)
    return out.reshape(B, S, H).astype(np.float32)
